# revision 22
# baseline (speedup 1.0000x reference)
"""Trainium2 Bass kernel for nn_DynamicFilter (dynamic per-image 3x3 grouped filter).

Math (per batch n, channel c, group g = c//4):
    pooled[n,c] = mean_hw x[n,c]
    f = pooled @ W2.T + b2          (conv1x1 + folded BN)
    filt[n,g,k] = tanh(f)           (k = 9 taps, 3x3, reflect pad)
    out = A_c * conv3x3_{filt[g]}(x) + s_c * x + Bc_c * pooled[n,c]
      A = lamb_l*(inside_all+1), s = lamb_h+1, Bc = -lamb_l*inside_all

Sharding: 8 cores = (n in 0..3) x (channel half in 0..1), 16 channels/core.
No collectives: the pooled mean needs all 32 channels per n, so each core
side-loads the partner half's pixels in fp8 (pooling only; mean tolerance is
generous) and reduces everything on the otherwise-idle PE during the load.

Device mapping per core (bf16 conv data, fp8 pooling side-load, fp32 PSUM):
  - x rows (H) on SBUF partitions, one window-tile set per channel with
    1-row overlaps and reflection rows/cols materialized at load (88 rows).
  - pooling: mask-column matmuls stream the window tensor into a [1,512]
    PSUM row of (channel, j-phase) partial sums; one DVE reduce folds the
    phases. The fp8 side-load gets the same treatment with a ones column.
  - filt: pooled row -> PE transpose -> [33,1] column (bias row folded) ->
    1x1-conv matmul -> tanh -> PE broadcast down 128 partitions.
  - 3x3 conv = 3 matmuls per channel accumulating in PSUM: each lhsT is a
    tridiagonal [in_row x out_row] matrix carrying the 3 vertical taps for
    one horizontal shift dx; rhs free-dim offset provides dx.
  - residual s*x folded into the center tridiagonal diagonal as sigma=s/A;
    per-channel scale A and bias Bc*pooled applied at PSUM evacuation,
    alternating DVE / ScalarE per channel parity to balance engines.
"""

import numpy as np
import ml_dtypes

import concourse.bass as bass
import concourse.mybir as mybir
import concourse.tile as tile
from concourse import bacc, bass_utils

F32 = mybir.dt.float32
BF16 = mybir.dt.bfloat16
FP8 = mybir.dt.float8e4
NPBF16 = ml_dtypes.bfloat16
NPFP8 = mybir.dt.np(FP8)

N_B, C, H, W = 4, 32, 256, 256
CPC = 16   # channels per core
NCORES = 8
EPS = 1e-5

# window geometry: (main_src_row0, main_nrows, main_dst_part0,
#                   reflect_src_row, reflect_dst_part,
#                   pool_part0, pool_part1, out_row0, out_nrows, in_parts)
WINDOWS = [
    (0, 87, 1, 1, 0, 1, 87, 0, 86, 88),     # rows -1..86 (part0 = reflect row 1)
    (85, 87, 0, None, None, 1, 86, 86, 85, 87),
    (170, 86, 0, 254, 86, 1, 86, 171, 85, 87),  # part86 = reflect row 254
]
WBLK = 264  # column stride of one window block inside a channel tile
NPART = 88  # partitions carrying window rows
XEC = 8192  # side-load columns: (t=2, x=256, c=16)


def _build_nc():
    nc = bacc.Bacc(num_swdge_queues=4)
    xw = nc.declare_dram_parameter("xw", [NPART, 3 * CPC * WBLK], BF16, isOutput=False)
    xe = nc.declare_dram_parameter("xe", [128, XEC], FP8, isOutput=False)
    xo = nc.declare_dram_parameter("xo", [128, XEC], FP8, isOutput=False)
    w2t = nc.declare_dram_parameter("w2t", [33, 36], BF16, isOutput=False)
    bctc = nc.declare_dram_parameter("bctc", [128, 48], F32, isOutput=False)
    shifts = nc.declare_dram_parameter("shifts", [NPART, NPART], BF16, isOutput=False)
    maskcol = nc.declare_dram_parameter("maskcol", [NPART, 4], BF16, isOutput=False)
    out_d = nc.declare_dram_parameter("out", [86, 3 * CPC * W], BF16, isOutput=True)

    OBLK = 3 * W  # out-tile columns per channel (w0|w1|w2)

    with tile.TileContext(nc) as tc:
        with (
            tc.tile_pool(name="wbig", bufs=1) as wpool,
            tc.tile_pool(name="consts", bufs=1) as cpool,
            tc.tile_pool(name="small", bufs=1) as spool,
            tc.tile_pool(name="tri", bufs=1) as tripool,
            tc.tile_pool(name="tritmp", bufs=2) as tmppool,
            tc.tile_pool(name="outs", bufs=1) as opool,
            tc.tile_pool(name="ps_row", bufs=1, space="PSUM") as psr_pool,
            tc.tile_pool(name="ps_a", bufs=4, space="PSUM") as psa_pool,
            tc.tile_pool(name="ps_b", bufs=2, space="PSUM") as psb_pool,
        ):
            def on_q(inst, qn):
                if qn:
                    inst.ins.queue = f"qPoolDynamic{qn}"
                return inst

            # constants (HWDGE, parallel with the window loads)
            shifts_t = cpool.tile([NPART, NPART], BF16, tag="shifts")
            nc.sync.dma_start(shifts_t[:, :], shifts[:, :])
            w2t_t = cpool.tile([33, 36], BF16, tag="w2t")
            nc.sync.dma_start(w2t_t[:, :], w2t[:, :])
            maskcol_t = cpool.tile([NPART, 4], BF16, tag="maskcol")
            nc.sync.dma_start(maskcol_t[:, :], maskcol[:, :])
            bctc_t = cpool.tile([128, 48], F32, tag="bctc")
            nc.sync.dma_start(bctc_t[:, :], bctc[:, :])
            onesrow = cpool.tile([1, 128], BF16, tag="onesrow")
            nc.vector.memset(onesrow[:, :], 1.0)
            ones1 = cpool.tile([1, 1], BF16, tag="ones1")
            nc.vector.memset(ones1[:, :], 1.0)
            onescol8 = cpool.tile([128, 1], FP8, tag="onescol8")
            nc.vector.memset(onescol8[:, :], 1.0)
            pr = cpool.tile([1, 33], BF16, tag="pr")
            nc.vector.memset(pr[0:1, 32:33], 1.0)

            # compact fp8 side-loads of BOTH channel halves (pooling only):
            # pooled/filt completes while the bf16 windows stream in.
            xots, xets = [], []
            for k in range(2):
                xo_t = wpool.tile([128, XEC // 2], FP8, tag=f"xot{k}", name=f"xot{k}")
                on_q(nc.gpsimd.dma_start(
                    xo_t[:, :], xo[:, k * (XEC // 2):(k + 1) * (XEC // 2)]), k)
                xots.append(xo_t[:, :].rearrange("p (x c) -> p x c", c=CPC))
            for k in range(2):
                xt = wpool.tile([128, XEC // 2], FP8, tag=f"xet{k}", name=f"xet{k}")
                on_q(nc.gpsimd.dma_start(
                    xt[:, :], xe[:, k * (XEC // 2):(k + 1) * (XEC // 2)]), 2 + k)
                xets.append(xt[:, :].rearrange("p (x c) -> p x c", c=CPC))

            # window tensor (h, w, c8, x): 8 flat pieces, h0 first
            wcvt = wpool.tile([NPART, 3 * CPC * WBLK], BF16, tag="wcvt")
            t4 = wcvt[:, :].rearrange("p (h w c x) -> p h w c x", h=2, w=3, x=WBLK)
            QB = 3 * 8 * WBLK // 4
            for k in [0, 1, 2, 3, 4, 5, 6, 7]:
                sl = slice(k * QB, (k + 1) * QB)
                if k % 4 == 0:
                    nc.sync.dma_start(wcvt[0:NPART, sl], xw[:, sl])
                elif k % 4 == 1:
                    nc.scalar.dma_start(wcvt[0:NPART, sl], xw[:, sl])
                else:
                    on_q(nc.gpsimd.dma_start(wcvt[0:NPART, sl], xw[:, sl]),
                         (k % 4) - 2 + (2 if k >= 4 else 0))

            with nc.allow_low_precision(reason="bf16/fp8 pooled sums"):
                # partner pooling: ones-column matmuls into a (x-phase, c) row
                pse = psr_pool.tile([1, 512], F32, tag="pse")
                for ki in range(16):
                    piece, jh = divmod(ki, 8)
                    nc.tensor.matmul(
                        pse[:, :],
                        onescol8[:, :],
                        xets[piece][0:128, jh * 32:(jh + 1) * 32, 0:CPC],
                        start=(ki == 0),
                        stop=(ki == 15),
                    )
                # own pooling: same ones-column scheme over the compact copy
                pso = psr_pool.tile([1, 512], F32, tag="pso")
                for ki in range(16):
                    piece, jh = divmod(ki, 8)
                    nc.tensor.matmul(
                        pso[:, :],
                        onescol8[:, :],
                        xots[piece][0:128, jh * 32:(jh + 1) * 32, 0:CPC],
                        start=(ki == 0),
                        stop=(ki == 15),
                    )
                # fold the phases: pr = [own16 | partner16 | 1.0]
                nc.vector.tensor_reduce(
                    pr[0:1, 0:16],
                    pso[0:1, :].rearrange("p (j c) -> p c j", c=CPC),
                    axis=mybir.AxisListType.X, op=mybir.AluOpType.add,
                )
                nc.vector.tensor_reduce(
                    pr[0:1, 16:32],
                    pse[0:1, :].rearrange("p (j c) -> p c j", c=CPC),
                    axis=mybir.AxisListType.X, op=mybir.AluOpType.add,
                )

                # one transpose: [own16 | partner16 | 1.0] row -> [33,1] column
                pcl = psr_pool.tile([33, 1], BF16, tag="pse", name="pcl")
                nc.tensor.transpose(pcl[:, :], pr[0:1, :], ones1[:, :])
                pcol = spool.tile([33, 1], BF16, tag="pcol")
                nc.scalar.activation(pcol[:, :], pcl[:, :],
                                     mybir.ActivationFunctionType.Copy)

                # f = pooled @ W2s.T + b2 ; filt = tanh(f)
                f_ps = psr_pool.tile([1, 36], F32, tag="pso", name="f_ps")
                nc.tensor.matmul(f_ps[:, :], pcol[:, :], w2t_t[:, :])
                filtrow = spool.tile([1, 36], BF16, tag="filtrow")
                nc.scalar.activation(filtrow[:, :], f_ps[:, :],
                                     mybir.ActivationFunctionType.Tanh)

                # broadcast filt down 128 partitions; Bc*pooled likewise
                fbc_ps = psr_pool.tile([128, 36], F32, tag="pse", name="fbc_ps")
                nc.tensor.matmul(fbc_ps[:, :], onesrow[:, :], filtrow[:, :])
                fbc = spool.tile([128, 36], F32, tag="fbc")
                nc.scalar.activation(fbc[:, :], fbc_ps[:, :],
                                     mybir.ActivationFunctionType.Copy)
                pbc_ps = psr_pool.tile([128, 16], F32, tag="pse", name="pbc_ps")
                nc.tensor.matmul(pbc_ps[:, :], onesrow[:, :], pr[0:1, 0:16])
                bp = spool.tile([128, 16], F32, tag="bp")
                nc.vector.tensor_mul(bp[:, :], bctc_t[0:128, 32:48], pbc_ps[:, :])


            # main conv + evac per channel; tridiag builds interleaved so
            # the DVE FIFO isn't clogged ahead of the evacuations.
            ot = opool.tile([86, 3 * CPC * W], BF16, tag="ot")
            ot4 = ot[:, :].rearrange("p (c w x) -> p c w x", w=3, x=W)
            traw = {}
            tc0 = {}
            with nc.allow_low_precision(reason="bf16 conv pipeline"):
                for ch in range(CPC):
                    g = ch // 4
                    if ch % 4 == 0:
                        for dxi, dx in enumerate((-1, 0, 1)):
                            wm = 9 * g + (dx + 1)
                            w0c = wm + 3
                            wp = wm + 6
                            t1 = tmppool.tile([NPART, 86], BF16, tag="t1")
                            nc.vector.tensor_scalar_mul(
                                t1[:, :], shifts_t[0:NPART, 0:86], fbc[0:NPART, wm:wm + 1]
                            )
                            t2 = tmppool.tile([NPART, 86], BF16, tag="t2")
                            nc.vector.scalar_tensor_tensor(
                                t2[:, :], shifts_t[0:NPART, 1:87], fbc[0:NPART, w0c:w0c + 1],
                                t1[:, :], op0=mybir.AluOpType.mult, op1=mybir.AluOpType.add,
                            )
                            tr = tripool.tile([NPART, 86], BF16, tag=f"traw{g}_{dxi}")
                            nc.vector.scalar_tensor_tensor(
                                tr[:, :], shifts_t[0:NPART, 2:88], fbc[0:NPART, wp:wp + 1],
                                t2[:, :], op0=mybir.AluOpType.mult, op1=mybir.AluOpType.add,
                            )
                            traw[(g, dxi)] = tr
                    t = tripool.tile([NPART, 86], BF16, tag=f"tc0_{ch}")
                    nc.vector.scalar_tensor_tensor(
                        t[:, :], shifts_t[0:NPART, 1:87],
                        bctc_t[0:NPART, 16 + ch:16 + ch + 1],
                        traw[(g, 1)][:, :], op0=mybir.AluOpType.mult,
                        op1=mybir.AluOpType.add,
                    )
                    tc0[ch] = t
                    psa = psa_pool.tile([86, 512], F32, tag="psa")
                    if ch % 2 == 0:
                        psb2 = psb_pool.tile([85, 512], F32, tag="psb")
                        psb2_saved = psb2
                    else:
                        psb2 = psb2_saved
                    half = (ch % 2) * W
                    for dxi, dx in enumerate((-1, 0, 1)):
                        lt = tc0[ch] if dx == 0 else traw[(g, dxi)]
                        hh, c8 = divmod(ch, 8)
                        nc.tensor.matmul(
                            psa[:, :],
                            lt[0:NPART, 0:86],
                            t4[0:NPART, hh, 0:2, c8, dx + 1:dx + 257],
                            start=(dxi == 0),
                            stop=(dxi == 2),
                        )
                        if dx == 0:
                            nc.tensor.matmul(
                                psb2[0:85, half:half + W],
                                lt[0:87, 0:85],
                                t4[0:87, hh, 2, c8, dx + 1:dx + 257],
                                start=False,
                                stop=(ch % 2 == 1),
                            )
                        elif ch % 2 == 0:
                            # paired: w2 of ch and ch+1 share the group lhsT
                            nc.tensor.matmul(
                                psb2[0:85, :],
                                lt[0:87, 0:85],
                                t4[0:87, hh, 2, c8:c8 + 2, dx + 1:dx + 257],
                                start=(dxi == 0),
                                stop=False,
                            )
                    # psa evac alternates DVE / ScalarE to balance engines
                    if ch % 2 == 0:
                        nc.vector.tensor_scalar(
                            ot4[0:86, ch, 0:2, :],
                            psa[:, :].rearrange("p (a b) -> p a b", b=W),
                            bctc_t[0:86, ch:ch + 1],
                            bp[0:86, ch:ch + 1],
                            op0=mybir.AluOpType.mult,
                            op1=mybir.AluOpType.add,
                        )
                    else:
                        nc.scalar.activation(
                            ot4[0:86, ch, 0:2, :],
                            psa[:, :].rearrange("p (a b) -> p a b", b=W),
                            mybir.ActivationFunctionType.Identity,
                            bias=bp[0:86, ch:ch + 1],
                            scale=bctc_t[0:86, ch:ch + 1],
                        )
                    if ch % 2 == 1:
                        for c2 in (ch - 1, ch):
                            h2 = (c2 % 2) * W
                            nc.scalar.activation(
                                ot4[0:85, c2, 2, :], psb2[0:85, h2:h2 + W],
                                mybir.ActivationFunctionType.Identity,
                                bias=bp[0:85, c2:c2 + 1],
                                scale=bctc_t[0:85, c2:c2 + 1],
                            )
                        # out DMA per channel pair on the HWDGE rings so the
                        # SWDGE drain at kernel end has nothing left to flush
                        p0 = ch - 1
                        c0 = p0 * OBLK
                        ring = nc.sync if (p0 // 2) % 2 == 0 else nc.scalar
                        ring.dma_start(
                            out_d[0:86, c0:c0 + 2 * OBLK],
                            ot[0:86, c0:c0 + 2 * OBLK],
                        )

    nc.compile()
    return nc


_NC_CACHE = None


def _get_nc():
    global _NC_CACHE
    if _NC_CACHE is None:
        _NC_CACHE = _build_nc()
    return _NC_CACHE


def _maskcol_np():
    s = np.zeros((NPART, 4), np.float32)
    for wi, win in enumerate(WINDOWS):
        pv0, pv1 = win[5], win[6]
        s[pv0:pv1, wi] = 1.0
    return s.astype(NPBF16)


def _shifts_np():
    return np.eye(NPART, dtype=np.float32).astype(NPBF16)


# row indices per window (length NPART; tail rows unused -> clamp to 0)
def _win_rows():
    rows = []
    for wi, (r0, nr, p0, rr, rp, _, _, _, _, nparts) in enumerate(WINDOWS):
        idx = np.zeros(NPART, np.int64)
        idx[p0:p0 + nr] = np.arange(r0, r0 + nr)
        if rr is not None:
            idx[rp] = rr
        rows.append((idx, nparts))
    return rows


_WIN_ROWS = _win_rows()


def _build_windows(xs_np):
    """xs_np [16, 256, 256] fp32 -> [NPART, (h2, w3, c8, 264)] bf16 windows."""
    out = np.zeros((NPART, 3, CPC, WBLK), NPBF16)
    xb = xs_np.astype(NPBF16)
    for wi, (idx, nparts) in enumerate(_WIN_ROWS):
        g = xb[:, idx[:nparts], :]             # [16, nparts, 256]
        g = np.ascontiguousarray(g.transpose(1, 0, 2))  # [nparts, 16, 256]
        out[:nparts, wi, :, 1:257] = g
        out[:nparts, wi, :, 0] = g[:, :, 1]
        out[:nparts, wi, :, 257] = g[:, :, 254]
    out = out.reshape(NPART, 3, 2, 8, WBLK).transpose(0, 2, 1, 3, 4)
    return np.ascontiguousarray(out).reshape(NPART, 3 * CPC * WBLK)


def _build_xe(xs_np):
    """xs_np [16, 256, 256] fp32 -> [128, (t=2, x=256, c=16)] fp8."""
    r = xs_np.reshape(CPC, 2, 128, 256).transpose(2, 1, 3, 0)  # [128, 2, 256, 16]
    return np.ascontiguousarray(r).astype(NPFP8).reshape(128, XEC)


def _scatter_out(flat, dst):
    """flat [86, 16*3*256] bf16 (c, w, x) -> dst [16, 256, 256] fp32."""
    f = flat.astype(np.float32).reshape(86, CPC, 3, W)
    dst[:, 0:86, :] = f[0:86, :, 0].transpose(1, 0, 2)
    dst[:, 86:171, :] = f[0:85, :, 1].transpose(1, 0, 2)
    dst[:, 171:256, :] = f[0:85, :, 2].transpose(1, 0, 2)


def kernel(x, conv_w, bn_gamma, bn_beta, bn_mean, bn_var, lamb_l, lamb_h, inside_all):
    x = np.asarray(x, np.float32)
    conv_w = np.asarray(conv_w, np.float32)
    bn_gamma = np.asarray(bn_gamma, np.float32)
    bn_beta = np.asarray(bn_beta, np.float32)
    bn_mean = np.asarray(bn_mean, np.float32)
    bn_var = np.asarray(bn_var, np.float32)
    lamb_l = np.asarray(lamb_l, np.float32)
    lamb_h = np.asarray(lamb_h, np.float32)
    ia = np.asarray(inside_all, np.float32).reshape(C)

    gv = (bn_gamma / np.sqrt(bn_var + np.float32(EPS))).astype(np.float32)
    w2s = (conv_w * gv[:, None] / np.float32(H * W)).astype(np.float32)  # [72, 32]
    b2 = (bn_beta - bn_mean * gv).astype(np.float32)                      # [72]

    A = (lamb_l * (ia + 1.0)).astype(np.float32)
    s = (lamb_h + 1.0).astype(np.float32)
    # device bias multiplies Bc by the pooled SUM, so fold the mean's 1/HW here
    Bc = (-lamb_l * ia / np.float32(H * W)).astype(np.float32)
    A_eff = np.where(A >= 0, np.maximum(A, 1e-20), np.minimum(A, -1e-20)).astype(np.float32)
    sig = (s / A_eff).astype(np.float32)

    shifts = _shifts_np()
    maskcol = _maskcol_np()
    nc = _get_nc()

    in_maps = []
    for core in range(NCORES):
        n = core // 2
        half = core % 2
        csl = slice(16 * half, 16 * half + 16)
        osl = slice(16 * (1 - half), 16 * (1 - half) + 16)
        gsl = slice(36 * half, 36 * half + 36)
        bctc_row = np.concatenate([A_eff[csl], sig[csl], Bc[csl]]).astype(np.float32)
        w2sT = np.ascontiguousarray(w2s[gsl].T)          # [32 in-ch, 36]
        w2t_full = np.concatenate(
            [w2sT[csl], w2sT[osl], b2[gsl].reshape(1, 36)], axis=0
        ).astype(NPBF16)                                  # [33, 36]
        in_maps.append({
            "xw": _build_windows(x[n, csl]),
            "xo": _build_xe(x[n, csl]),
            "xe": _build_xe(x[n, osl]),
            "w2t": w2t_full,
            "bctc": np.tile(bctc_row[None, :], (128, 1)),
            "shifts": shifts,
            "maskcol": maskcol,
        })

    res = bass_utils.run_bass_kernel_spmd(nc, in_maps, core_ids=list(range(NCORES)))

    out = np.empty((N_B, C, H, W), np.float32)
    for core in range(NCORES):
        n = core // 2
        half = core % 2
        _scatter_out(res.results[core]["out"], out[n, 16 * half:16 * half + 16])
    return out


# revision 24
# speedup vs baseline: 1.1426x; 1.1426x over previous
"""Trainium2 Bass kernel for nn_DynamicFilter (dynamic per-image 3x3 grouped filter).

Math (per batch n, channel c, group g = c//4):
    pooled[n,c] = mean_hw x[n,c]
    f = pooled @ W2.T + b2          (conv1x1 + folded BN)
    filt[n,g,k] = tanh(f)           (k = 9 taps, 3x3, reflect pad)
    out = A_c * conv3x3_{filt[g]}(x) + s_c * x + Bc_c * pooled[n,c]
      A = lamb_l*(inside_all+1), s = lamb_h+1, Bc = -lamb_l*inside_all

Sharding: 8 cores = (n in 0..3) x (channel half in 0..1), 16 channels/core.
No collectives: the pooled mean needs all 32 channels per n, so each core
side-loads the partner half's pixels in fp8 (pooling only; mean tolerance is
generous) and reduces everything on the otherwise-idle PE during the load.

Device mapping per core (bf16 conv data, fp8 pooling side-load, fp32 PSUM):
  - x rows (H) on SBUF partitions, one window-tile set per channel with
    1-row overlaps and reflection rows/cols materialized at load (88 rows).
  - pooling: mask-column matmuls stream the window tensor into a [1,512]
    PSUM row of (channel, j-phase) partial sums; one DVE reduce folds the
    phases. The fp8 side-load gets the same treatment with a ones column.
  - filt: pooled row -> PE transpose -> [33,1] column (bias row folded) ->
    1x1-conv matmul -> tanh -> PE broadcast down 128 partitions.
  - 3x3 conv = 3 matmuls per channel accumulating in PSUM: each lhsT is a
    tridiagonal [in_row x out_row] matrix carrying the 3 vertical taps for
    one horizontal shift dx; rhs free-dim offset provides dx.
  - residual s*x folded into the center tridiagonal diagonal as sigma=s/A;
    per-channel scale A and bias Bc*pooled applied at PSUM evacuation,
    alternating DVE / ScalarE per channel parity to balance engines.
"""

import numpy as np
import ml_dtypes

import concourse.bass as bass
import concourse.mybir as mybir
import concourse.tile as tile
from concourse import bacc, bass_utils

F32 = mybir.dt.float32
BF16 = mybir.dt.bfloat16
FP8 = mybir.dt.float8e4
NPBF16 = ml_dtypes.bfloat16
NPFP8 = mybir.dt.np(FP8)

N_B, C, H, W = 4, 32, 256, 256
CPC = 16   # channels per core
NCORES = 8
EPS = 1e-5

# window geometry: (main_src_row0, main_nrows, main_dst_part0,
#                   reflect_src_row, reflect_dst_part,
#                   pool_part0, pool_part1, out_row0, out_nrows, in_parts)
WINDOWS = [
    (0, 87, 1, 1, 0, 1, 87, 0, 86, 88),     # rows -1..86 (part0 = reflect row 1)
    (85, 87, 0, None, None, 1, 86, 86, 85, 87),
    (170, 86, 0, 254, 86, 1, 86, 171, 85, 87),  # part86 = reflect row 254
]
WBLK = 264  # column stride of one window block inside a channel tile
NPART = 88  # partitions carrying window rows
XEC = 8192  # side-load columns: (t=2, x=256, c=16)


def _build_nc():
    nc = bacc.Bacc(num_swdge_queues=4)
    xw = nc.declare_dram_parameter("xw", [NPART, 3 * CPC * WBLK], BF16, isOutput=False)
    xe = nc.declare_dram_parameter("xe", [128, XEC], FP8, isOutput=False)
    xo = nc.declare_dram_parameter("xo", [128, XEC], FP8, isOutput=False)
    w2t = nc.declare_dram_parameter("w2t", [33, 36], BF16, isOutput=False)
    bctc = nc.declare_dram_parameter("bctc", [128, 48], F32, isOutput=False)
    shifts = nc.declare_dram_parameter("shifts", [NPART, NPART], BF16, isOutput=False)
    maskcol = nc.declare_dram_parameter("maskcol", [NPART, 4], BF16, isOutput=False)
    out_d = nc.declare_dram_parameter("out", [86, 3 * CPC * W], BF16, isOutput=True)

    OBLK = 3 * W  # out-tile columns per channel (w0|w1|w2)

    with tile.TileContext(nc) as tc:
        with (
            tc.tile_pool(name="wbig", bufs=1) as wpool,
            tc.tile_pool(name="consts", bufs=1) as cpool,
            tc.tile_pool(name="small", bufs=1) as spool,
            tc.tile_pool(name="tri", bufs=1) as tripool,
            tc.tile_pool(name="tritmp", bufs=2) as tmppool,
            tc.tile_pool(name="outs", bufs=1) as opool,
            tc.tile_pool(name="ps_row", bufs=1, space="PSUM") as psr_pool,
            tc.tile_pool(name="ps_a", bufs=4, space="PSUM") as psa_pool,
            tc.tile_pool(name="ps_b", bufs=2, space="PSUM") as psb_pool,
        ):
            def on_q(inst, qn):
                if qn:
                    inst.ins.queue = f"qPoolDynamic{qn}"
                return inst

            # constants (HWDGE, parallel with the window loads)
            shifts_t = cpool.tile([NPART, NPART], BF16, tag="shifts")
            nc.sync.dma_start(shifts_t[:, :], shifts[:, :])
            w2t_t = cpool.tile([33, 36], BF16, tag="w2t")
            nc.sync.dma_start(w2t_t[:, :], w2t[:, :])
            maskcol_t = cpool.tile([NPART, 4], BF16, tag="maskcol")
            nc.sync.dma_start(maskcol_t[:, :], maskcol[:, :])
            bctc_t = cpool.tile([128, 48], F32, tag="bctc")
            nc.sync.dma_start(bctc_t[:, :], bctc[:, :])
            onesrow = cpool.tile([1, 128], BF16, tag="onesrow")
            nc.vector.memset(onesrow[:, :], 1.0)
            ones1 = cpool.tile([1, 1], BF16, tag="ones1")
            nc.vector.memset(ones1[:, :], 1.0)
            onescol8 = cpool.tile([128, 1], FP8, tag="onescol8")
            nc.vector.memset(onescol8[:, :], 1.0)
            pr = cpool.tile([1, 33], BF16, tag="pr")
            nc.vector.memset(pr[0:1, 32:33], 1.0)

            # compact fp8 side-loads of BOTH channel halves (pooling only):
            # pooled/filt completes while the bf16 windows stream in.
            xots, xets = [], []
            for k in range(2):
                xo_t = wpool.tile([128, XEC // 2], FP8, tag=f"xot{k}", name=f"xot{k}")
                on_q(nc.gpsimd.dma_start(
                    xo_t[:, :], xo[:, k * (XEC // 2):(k + 1) * (XEC // 2)]), k)
                xots.append(xo_t[:, :].rearrange("p (x c) -> p x c", c=CPC))
            for k in range(2):
                xt = wpool.tile([128, XEC // 2], FP8, tag=f"xet{k}", name=f"xet{k}")
                on_q(nc.gpsimd.dma_start(
                    xt[:, :], xe[:, k * (XEC // 2):(k + 1) * (XEC // 2)]), 2 + k)
                xets.append(xt[:, :].rearrange("p (x c) -> p x c", c=CPC))

            # window tensor (h, w, c8, x): 8 flat pieces, h0 first
            wcvt = wpool.tile([NPART, 3 * CPC * WBLK], BF16, tag="wcvt")
            t4 = wcvt[:, :].rearrange("p (h w c x) -> p h w c x", h=2, w=3, x=WBLK)
            QB = 3 * 8 * WBLK // 4
            for k in [0, 1, 2, 3, 4, 5, 6, 7]:
                sl = slice(k * QB, (k + 1) * QB)
                if k % 4 == 0:
                    nc.sync.dma_start(wcvt[0:NPART, sl], xw[:, sl])
                elif k % 4 == 1:
                    nc.scalar.dma_start(wcvt[0:NPART, sl], xw[:, sl])
                else:
                    on_q(nc.gpsimd.dma_start(wcvt[0:NPART, sl], xw[:, sl]),
                         (k % 4) - 2 + (2 if k >= 4 else 0))

            with nc.allow_low_precision(reason="bf16/fp8 pooled sums"):
                # junk matmuls on the constants warm the PE clock-gate before
                # the pooling stream arrives (their bank is cleared by pse's
                # start=True); ~3.5us of activity flips HAM to full rate
                wu = psr_pool.tile([1, 512], F32, tag="pse", name="wu")
                for i in range(24):
                    nc.tensor.matmul(
                        wu[0:1, 0:NPART],
                        shifts_t[0:NPART, i % 2:i % 2 + 1],
                        shifts_t[0:NPART, 0:NPART],
                        start=(i == 0),
                        stop=(i == 23),
                    )
                # partner pooling: ones-column matmuls into a (x-phase, c) row
                pse = psr_pool.tile([1, 512], F32, tag="pse")
                for ki in range(16):
                    piece, jh = divmod(ki, 8)
                    nc.tensor.matmul(
                        pse[:, :],
                        onescol8[:, :],
                        xets[piece][0:128, jh * 32:(jh + 1) * 32, 0:CPC],
                        start=(ki == 0),
                        stop=(ki == 15),
                    )
                # own pooling: same ones-column scheme over the compact copy
                pso = psr_pool.tile([1, 512], F32, tag="pso")
                for ki in range(16):
                    piece, jh = divmod(ki, 8)
                    nc.tensor.matmul(
                        pso[:, :],
                        onescol8[:, :],
                        xots[piece][0:128, jh * 32:(jh + 1) * 32, 0:CPC],
                        start=(ki == 0),
                        stop=(ki == 15),
                    )
                # fold the phases: pr = [own16 | partner16 | 1.0]
                nc.vector.tensor_reduce(
                    pr[0:1, 0:16],
                    pso[0:1, :].rearrange("p (j c) -> p c j", c=CPC),
                    axis=mybir.AxisListType.X, op=mybir.AluOpType.add,
                )
                nc.vector.tensor_reduce(
                    pr[0:1, 16:32],
                    pse[0:1, :].rearrange("p (j c) -> p c j", c=CPC),
                    axis=mybir.AxisListType.X, op=mybir.AluOpType.add,
                )

                # one transpose: [own16 | partner16 | 1.0] row -> [33,1] column
                pcl = psr_pool.tile([33, 1], BF16, tag="pse", name="pcl")
                nc.tensor.transpose(pcl[:, :], pr[0:1, :], ones1[:, :])
                pcol = spool.tile([33, 1], BF16, tag="pcol")
                nc.scalar.activation(pcol[:, :], pcl[:, :],
                                     mybir.ActivationFunctionType.Copy)

                # f = pooled @ W2s.T + b2 ; filt = tanh(f)
                f_ps = psr_pool.tile([1, 36], F32, tag="pso", name="f_ps")
                nc.tensor.matmul(f_ps[:, :], pcol[:, :], w2t_t[:, :])
                filtrow = spool.tile([1, 36], BF16, tag="filtrow")
                nc.scalar.activation(filtrow[:, :], f_ps[:, :],
                                     mybir.ActivationFunctionType.Tanh)

                # broadcast filt down 128 partitions; Bc*pooled likewise
                fbc_ps = psr_pool.tile([128, 36], F32, tag="pse", name="fbc_ps")
                nc.tensor.matmul(fbc_ps[:, :], onesrow[:, :], filtrow[:, :])
                fbc = spool.tile([128, 36], F32, tag="fbc")
                nc.scalar.activation(fbc[:, :], fbc_ps[:, :],
                                     mybir.ActivationFunctionType.Copy)
                pbc_ps = psr_pool.tile([128, 16], F32, tag="pse", name="pbc_ps")
                nc.tensor.matmul(pbc_ps[:, :], onesrow[:, :], pr[0:1, 0:16])
                bp = spool.tile([128, 16], F32, tag="bp")
                nc.vector.tensor_mul(bp[:, :], bctc_t[0:128, 32:48], pbc_ps[:, :])


            # main conv + evac per channel; tridiag builds interleaved so
            # the DVE FIFO isn't clogged ahead of the evacuations.
            ot = opool.tile([86, 3 * CPC * W], BF16, tag="ot")
            ot4 = ot[:, :].rearrange("p (c w x) -> p c w x", w=3, x=W)
            traw = {}
            tc0 = {}
            with nc.allow_low_precision(reason="bf16 conv pipeline"):
                for ch in range(CPC):
                    g = ch // 4
                    if ch % 4 == 0:
                        for dxi, dx in enumerate((-1, 0, 1)):
                            wm = 9 * g + (dx + 1)
                            w0c = wm + 3
                            wp = wm + 6
                            t1 = tmppool.tile([NPART, 86], BF16, tag="t1")
                            nc.vector.tensor_scalar_mul(
                                t1[:, :], shifts_t[0:NPART, 0:86], fbc[0:NPART, wm:wm + 1]
                            )
                            t2 = tmppool.tile([NPART, 86], BF16, tag="t2")
                            nc.vector.scalar_tensor_tensor(
                                t2[:, :], shifts_t[0:NPART, 1:87], fbc[0:NPART, w0c:w0c + 1],
                                t1[:, :], op0=mybir.AluOpType.mult, op1=mybir.AluOpType.add,
                            )
                            tr = tripool.tile([NPART, 86], BF16, tag=f"traw{g}_{dxi}")
                            nc.vector.scalar_tensor_tensor(
                                tr[:, :], shifts_t[0:NPART, 2:88], fbc[0:NPART, wp:wp + 1],
                                t2[:, :], op0=mybir.AluOpType.mult, op1=mybir.AluOpType.add,
                            )
                            traw[(g, dxi)] = tr
                    t = tripool.tile([NPART, 86], BF16, tag=f"tc0_{ch}")
                    nc.vector.scalar_tensor_tensor(
                        t[:, :], shifts_t[0:NPART, 1:87],
                        bctc_t[0:NPART, 16 + ch:16 + ch + 1],
                        traw[(g, 1)][:, :], op0=mybir.AluOpType.mult,
                        op1=mybir.AluOpType.add,
                    )
                    tc0[ch] = t
                    psa = psa_pool.tile([86, 512], F32, tag="psa")
                    if ch % 2 == 0:
                        psb2 = psb_pool.tile([85, 512], F32, tag="psb")
                        psb2_saved = psb2
                    else:
                        psb2 = psb2_saved
                    half = (ch % 2) * W
                    for dxi, dx in enumerate((-1, 0, 1)):
                        lt = tc0[ch] if dx == 0 else traw[(g, dxi)]
                        hh, c8 = divmod(ch, 8)
                        nc.tensor.matmul(
                            psa[:, :],
                            lt[0:NPART, 0:86],
                            t4[0:NPART, hh, 0:2, c8, dx + 1:dx + 257],
                            start=(dxi == 0),
                            stop=(dxi == 2),
                        )
                        if dx == 0:
                            nc.tensor.matmul(
                                psb2[0:85, half:half + W],
                                lt[0:87, 0:85],
                                t4[0:87, hh, 2, c8, dx + 1:dx + 257],
                                start=False,
                                stop=(ch % 2 == 1),
                            )
                        elif ch % 2 == 0:
                            # paired: w2 of ch and ch+1 share the group lhsT
                            nc.tensor.matmul(
                                psb2[0:85, :],
                                lt[0:87, 0:85],
                                t4[0:87, hh, 2, c8:c8 + 2, dx + 1:dx + 257],
                                start=(dxi == 0),
                                stop=False,
                            )
                    # psa evac alternates DVE / ScalarE to balance engines
                    if ch % 2 == 0:
                        nc.vector.tensor_scalar(
                            ot4[0:86, ch, 0:2, :],
                            psa[:, :].rearrange("p (a b) -> p a b", b=W),
                            bctc_t[0:86, ch:ch + 1],
                            bp[0:86, ch:ch + 1],
                            op0=mybir.AluOpType.mult,
                            op1=mybir.AluOpType.add,
                        )
                    else:
                        nc.scalar.activation(
                            ot4[0:86, ch, 0:2, :],
                            psa[:, :].rearrange("p (a b) -> p a b", b=W),
                            mybir.ActivationFunctionType.Identity,
                            bias=bp[0:86, ch:ch + 1],
                            scale=bctc_t[0:86, ch:ch + 1],
                        )
                    if ch % 2 == 1:
                        for c2 in (ch - 1, ch):
                            h2 = (c2 % 2) * W
                            nc.scalar.activation(
                                ot4[0:85, c2, 2, :], psb2[0:85, h2:h2 + W],
                                mybir.ActivationFunctionType.Identity,
                                bias=bp[0:85, c2:c2 + 1],
                                scale=bctc_t[0:85, c2:c2 + 1],
                            )
                        # out DMA per channel pair
                        p0 = ch - 1
                        c0 = p0 * OBLK
                        on_q(nc.gpsimd.dma_start(
                            out_d[0:86, c0:c0 + 2 * OBLK],
                            ot[0:86, c0:c0 + 2 * OBLK],
                        ), (p0 // 2) % 4)

    nc.compile()
    return nc


_NC_CACHE = None


def _get_nc():
    global _NC_CACHE
    if _NC_CACHE is None:
        _NC_CACHE = _build_nc()
    return _NC_CACHE


def _maskcol_np():
    s = np.zeros((NPART, 4), np.float32)
    for wi, win in enumerate(WINDOWS):
        pv0, pv1 = win[5], win[6]
        s[pv0:pv1, wi] = 1.0
    return s.astype(NPBF16)


def _shifts_np():
    return np.eye(NPART, dtype=np.float32).astype(NPBF16)


# row indices per window (length NPART; tail rows unused -> clamp to 0)
def _win_rows():
    rows = []
    for wi, (r0, nr, p0, rr, rp, _, _, _, _, nparts) in enumerate(WINDOWS):
        idx = np.zeros(NPART, np.int64)
        idx[p0:p0 + nr] = np.arange(r0, r0 + nr)
        if rr is not None:
            idx[rp] = rr
        rows.append((idx, nparts))
    return rows


_WIN_ROWS = _win_rows()


def _build_windows(xs_np):
    """xs_np [16, 256, 256] fp32 -> [NPART, (h2, w3, c8, 264)] bf16 windows."""
    out = np.zeros((NPART, 3, CPC, WBLK), NPBF16)
    xb = xs_np.astype(NPBF16)
    for wi, (idx, nparts) in enumerate(_WIN_ROWS):
        g = xb[:, idx[:nparts], :]             # [16, nparts, 256]
        g = np.ascontiguousarray(g.transpose(1, 0, 2))  # [nparts, 16, 256]
        out[:nparts, wi, :, 1:257] = g
        out[:nparts, wi, :, 0] = g[:, :, 1]
        out[:nparts, wi, :, 257] = g[:, :, 254]
    out = out.reshape(NPART, 3, 2, 8, WBLK).transpose(0, 2, 1, 3, 4)
    return np.ascontiguousarray(out).reshape(NPART, 3 * CPC * WBLK)


def _build_xe(xs_np):
    """xs_np [16, 256, 256] fp32 -> [128, (t=2, x=256, c=16)] fp8."""
    r = xs_np.reshape(CPC, 2, 128, 256).transpose(2, 1, 3, 0)  # [128, 2, 256, 16]
    return np.ascontiguousarray(r).astype(NPFP8).reshape(128, XEC)


def _scatter_out(flat, dst):
    """flat [86, 16*3*256] bf16 (c, w, x) -> dst [16, 256, 256] fp32."""
    f = flat.astype(np.float32).reshape(86, CPC, 3, W)
    dst[:, 0:86, :] = f[0:86, :, 0].transpose(1, 0, 2)
    dst[:, 86:171, :] = f[0:85, :, 1].transpose(1, 0, 2)
    dst[:, 171:256, :] = f[0:85, :, 2].transpose(1, 0, 2)


def kernel(x, conv_w, bn_gamma, bn_beta, bn_mean, bn_var, lamb_l, lamb_h, inside_all):
    x = np.asarray(x, np.float32)
    conv_w = np.asarray(conv_w, np.float32)
    bn_gamma = np.asarray(bn_gamma, np.float32)
    bn_beta = np.asarray(bn_beta, np.float32)
    bn_mean = np.asarray(bn_mean, np.float32)
    bn_var = np.asarray(bn_var, np.float32)
    lamb_l = np.asarray(lamb_l, np.float32)
    lamb_h = np.asarray(lamb_h, np.float32)
    ia = np.asarray(inside_all, np.float32).reshape(C)

    gv = (bn_gamma / np.sqrt(bn_var + np.float32(EPS))).astype(np.float32)
    w2s = (conv_w * gv[:, None] / np.float32(H * W)).astype(np.float32)  # [72, 32]
    b2 = (bn_beta - bn_mean * gv).astype(np.float32)                      # [72]

    A = (lamb_l * (ia + 1.0)).astype(np.float32)
    s = (lamb_h + 1.0).astype(np.float32)
    # device bias multiplies Bc by the pooled SUM, so fold the mean's 1/HW here
    Bc = (-lamb_l * ia / np.float32(H * W)).astype(np.float32)
    A_eff = np.where(A >= 0, np.maximum(A, 1e-20), np.minimum(A, -1e-20)).astype(np.float32)
    sig = (s / A_eff).astype(np.float32)

    shifts = _shifts_np()
    maskcol = _maskcol_np()
    nc = _get_nc()

    in_maps = []
    for core in range(NCORES):
        n = core // 2
        half = core % 2
        csl = slice(16 * half, 16 * half + 16)
        osl = slice(16 * (1 - half), 16 * (1 - half) + 16)
        gsl = slice(36 * half, 36 * half + 36)
        bctc_row = np.concatenate([A_eff[csl], sig[csl], Bc[csl]]).astype(np.float32)
        w2sT = np.ascontiguousarray(w2s[gsl].T)          # [32 in-ch, 36]
        w2t_full = np.concatenate(
            [w2sT[csl], w2sT[osl], b2[gsl].reshape(1, 36)], axis=0
        ).astype(NPBF16)                                  # [33, 36]
        in_maps.append({
            "xw": _build_windows(x[n, csl]),
            "xo": _build_xe(x[n, csl]),
            "xe": _build_xe(x[n, osl]),
            "w2t": w2t_full,
            "bctc": np.tile(bctc_row[None, :], (128, 1)),
            "shifts": shifts,
            "maskcol": maskcol,
        })

    res = bass_utils.run_bass_kernel_spmd(nc, in_maps, core_ids=list(range(NCORES)))

    out = np.empty((N_B, C, H, W), np.float32)
    for core in range(NCORES):
        n = core // 2
        half = core % 2
        _scatter_out(res.results[core]["out"], out[n, 16 * half:16 * half + 16])
    return out


# revision 25
# speedup vs baseline: 1.1671x; 1.0215x over previous
"""Trainium2 Bass kernel for nn_DynamicFilter (dynamic per-image 3x3 grouped filter).

Math (per batch n, channel c, group g = c//4):
    pooled[n,c] = mean_hw x[n,c]
    f = pooled @ W2.T + b2          (conv1x1 + folded BN)
    filt[n,g,k] = tanh(f)           (k = 9 taps, 3x3, reflect pad)
    out = A_c * conv3x3_{filt[g]}(x) + s_c * x + Bc_c * pooled[n,c]
      A = lamb_l*(inside_all+1), s = lamb_h+1, Bc = -lamb_l*inside_all

Sharding: 8 cores = (n in 0..3) x (channel half in 0..1), 16 channels/core.
No collectives: the pooled mean needs all 32 channels per n, so each core
side-loads the partner half's pixels in fp8 (pooling only; mean tolerance is
generous) and reduces everything on the otherwise-idle PE during the load.

Device mapping per core (bf16 conv data, fp8 pooling side-load, fp32 PSUM):
  - x rows (H) on SBUF partitions, one window-tile set per channel with
    1-row overlaps and reflection rows/cols materialized at load (88 rows).
  - pooling: mask-column matmuls stream the window tensor into a [1,512]
    PSUM row of (channel, j-phase) partial sums; one DVE reduce folds the
    phases. The fp8 side-load gets the same treatment with a ones column.
  - filt: pooled row -> PE transpose -> [33,1] column (bias row folded) ->
    1x1-conv matmul -> tanh -> PE broadcast down 128 partitions.
  - 3x3 conv = 3 matmuls per channel accumulating in PSUM: each lhsT is a
    tridiagonal [in_row x out_row] matrix carrying the 3 vertical taps for
    one horizontal shift dx; rhs free-dim offset provides dx.
  - residual s*x folded into the center tridiagonal diagonal as sigma=s/A;
    per-channel scale A and bias Bc*pooled applied at PSUM evacuation,
    alternating DVE / ScalarE per channel parity to balance engines.
"""

import numpy as np
import ml_dtypes

import concourse.bass as bass
import concourse.mybir as mybir
import concourse.tile as tile
from concourse import bacc, bass_utils

F32 = mybir.dt.float32
BF16 = mybir.dt.bfloat16
FP8 = mybir.dt.float8e4
NPBF16 = ml_dtypes.bfloat16
NPFP8 = mybir.dt.np(FP8)

N_B, C, H, W = 4, 32, 256, 256
CPC = 16   # channels per core
NCORES = 8
EPS = 1e-5

# window geometry: (main_src_row0, main_nrows, main_dst_part0,
#                   reflect_src_row, reflect_dst_part,
#                   pool_part0, pool_part1, out_row0, out_nrows, in_parts)
WINDOWS = [
    (0, 87, 1, 1, 0, 1, 87, 0, 86, 88),     # rows -1..86 (part0 = reflect row 1)
    (85, 87, 0, None, None, 1, 86, 86, 85, 87),
    (170, 86, 0, 254, 86, 1, 86, 171, 85, 87),  # part86 = reflect row 254
]
WBLK = 264  # column stride of one window block inside a channel tile
NPART = 88  # partitions carrying window rows
XEC = 8192  # side-load columns: (t=2, x=256, c=16)


def _build_nc():
    nc = bacc.Bacc(num_swdge_queues=4)
    xw = nc.declare_dram_parameter("xw", [NPART, 3 * CPC * WBLK], BF16, isOutput=False)
    xe = nc.declare_dram_parameter("xe", [128, XEC], FP8, isOutput=False)
    xo = nc.declare_dram_parameter("xo", [128, XEC], FP8, isOutput=False)
    w2t = nc.declare_dram_parameter("w2t", [33, 36], BF16, isOutput=False)
    bctc = nc.declare_dram_parameter("bctc", [128, 48], F32, isOutput=False)
    shifts = nc.declare_dram_parameter("shifts", [NPART, NPART], BF16, isOutput=False)
    maskcol = nc.declare_dram_parameter("maskcol", [NPART, 4], BF16, isOutput=False)
    out_d = nc.declare_dram_parameter("out", [86, 3 * CPC * W], BF16, isOutput=True)

    OBLK = 3 * W  # out-tile columns per channel (w0|w1|w2)

    with tile.TileContext(nc) as tc:
        with (
            tc.tile_pool(name="wbig", bufs=1) as wpool,
            tc.tile_pool(name="consts", bufs=1) as cpool,
            tc.tile_pool(name="small", bufs=1) as spool,
            tc.tile_pool(name="tri", bufs=1) as tripool,
            tc.tile_pool(name="tritmp", bufs=2) as tmppool,
            tc.tile_pool(name="outs", bufs=1) as opool,
            tc.tile_pool(name="ps_row", bufs=1, space="PSUM") as psr_pool,
            tc.tile_pool(name="ps_a", bufs=4, space="PSUM") as psa_pool,
            tc.tile_pool(name="ps_b", bufs=2, space="PSUM") as psb_pool,
        ):
            def on_q(inst, qn):
                if qn:
                    inst.ins.queue = f"qPoolDynamic{qn}"
                return inst

            # constants (HWDGE, parallel with the window loads)
            shifts_t = cpool.tile([NPART, NPART], BF16, tag="shifts")
            nc.sync.dma_start(shifts_t[:, :], shifts[:, :])
            w2t_t = cpool.tile([33, 36], BF16, tag="w2t")
            nc.sync.dma_start(w2t_t[:, :], w2t[:, :])
            maskcol_t = cpool.tile([NPART, 4], BF16, tag="maskcol")
            nc.sync.dma_start(maskcol_t[:, :], maskcol[:, :])
            bctc_t = cpool.tile([128, 48], F32, tag="bctc")
            nc.sync.dma_start(bctc_t[:, :], bctc[:, :])
            onesrow = cpool.tile([1, 128], BF16, tag="onesrow")
            nc.vector.memset(onesrow[:, :], 1.0)
            ones1 = cpool.tile([1, 1], BF16, tag="ones1")
            nc.vector.memset(ones1[:, :], 1.0)
            onescol8 = cpool.tile([128, 1], FP8, tag="onescol8")
            nc.vector.memset(onescol8[:, :], 1.0)
            pr = cpool.tile([1, 33], BF16, tag="pr")
            nc.vector.memset(pr[0:1, 32:33], 1.0)

            # compact fp8 side-loads of BOTH channel halves (pooling only):
            # pooled/filt completes while the bf16 windows stream in.
            xots, xets = [], []
            for k in range(2):
                xo_t = wpool.tile([128, XEC // 2], FP8, tag=f"xot{k}", name=f"xot{k}")
                on_q(nc.gpsimd.dma_start(
                    xo_t[:, :], xo[:, k * (XEC // 2):(k + 1) * (XEC // 2)]), k)
                xots.append(xo_t[:, :].rearrange("p (x c) -> p x c", c=CPC))
            for k in range(2):
                xt = wpool.tile([128, XEC // 2], FP8, tag=f"xet{k}", name=f"xet{k}")
                on_q(nc.gpsimd.dma_start(
                    xt[:, :], xe[:, k * (XEC // 2):(k + 1) * (XEC // 2)]), 2 + k)
                xets.append(xt[:, :].rearrange("p (x c) -> p x c", c=CPC))

            # window tensor (h, w, c8, x): 8 flat pieces, h0 first
            wcvt = wpool.tile([NPART, 3 * CPC * WBLK], BF16, tag="wcvt")
            t4 = wcvt[:, :].rearrange("p (h w c x) -> p h w c x", h=2, w=3, x=WBLK)
            QB = 3 * 8 * WBLK // 4
            for k in [0, 1, 2, 3, 4, 5, 6, 7]:
                sl = slice(k * QB, (k + 1) * QB)
                if k % 4 == 0:
                    nc.sync.dma_start(wcvt[0:NPART, sl], xw[:, sl])
                elif k % 4 == 1:
                    nc.scalar.dma_start(wcvt[0:NPART, sl], xw[:, sl])
                else:
                    on_q(nc.gpsimd.dma_start(wcvt[0:NPART, sl], xw[:, sl]),
                         (k % 4) - 2 + (2 if k >= 4 else 0))

            with nc.allow_low_precision(reason="bf16/fp8 pooled sums"):
                # junk matmuls on the constants warm the PE clock-gate before
                # the pooling stream arrives (their bank is cleared by pse's
                # start=True); ~3.5us of activity flips HAM to full rate
                wu = psr_pool.tile([1, 512], F32, tag="pse", name="wu")
                for i in range(24):
                    nc.tensor.matmul(
                        wu[0:1, 0:NPART],
                        shifts_t[0:NPART, i % 2:i % 2 + 1],
                        shifts_t[0:NPART, 0:NPART],
                        start=(i == 0),
                        stop=(i == 23),
                    )
                # partner pooling: ones-column matmuls into a (x-phase, c) row
                pse = psr_pool.tile([1, 512], F32, tag="pse")
                for ki in range(16):
                    piece, jh = divmod(ki, 8)
                    nc.tensor.matmul(
                        pse[:, :],
                        onescol8[:, :],
                        xets[piece][0:128, jh * 32:(jh + 1) * 32, 0:CPC],
                        start=(ki == 0),
                        stop=(ki == 15),
                    )
                # own pooling: same ones-column scheme over the compact copy
                pso = psr_pool.tile([1, 512], F32, tag="pso")
                for ki in range(16):
                    piece, jh = divmod(ki, 8)
                    nc.tensor.matmul(
                        pso[:, :],
                        onescol8[:, :],
                        xots[piece][0:128, jh * 32:(jh + 1) * 32, 0:CPC],
                        start=(ki == 0),
                        stop=(ki == 15),
                    )
                # fold the phases: pr = [own16 | partner16 | 1.0]
                nc.vector.tensor_reduce(
                    pr[0:1, 0:16],
                    pso[0:1, :].rearrange("p (j c) -> p c j", c=CPC),
                    axis=mybir.AxisListType.X, op=mybir.AluOpType.add,
                )
                nc.vector.tensor_reduce(
                    pr[0:1, 16:32],
                    pse[0:1, :].rearrange("p (j c) -> p c j", c=CPC),
                    axis=mybir.AxisListType.X, op=mybir.AluOpType.add,
                )

                # one transpose: [own16 | partner16 | 1.0] row -> [33,1] column
                pcl = psr_pool.tile([33, 1], BF16, tag="pse", name="pcl")
                nc.tensor.transpose(pcl[:, :], pr[0:1, :], ones1[:, :])
                pcol = spool.tile([33, 1], BF16, tag="pcol")
                nc.scalar.activation(pcol[:, :], pcl[:, :],
                                     mybir.ActivationFunctionType.Copy)

                # f = pooled @ W2s.T + b2 ; filt = tanh(f)
                f_ps = psr_pool.tile([1, 36], F32, tag="pso", name="f_ps")
                nc.tensor.matmul(f_ps[:, :], pcol[:, :], w2t_t[:, :])
                filtrow = spool.tile([1, 36], BF16, tag="filtrow")
                nc.scalar.activation(filtrow[:, :], f_ps[:, :],
                                     mybir.ActivationFunctionType.Tanh)

                # broadcast filt down 128 partitions; Bc*pooled likewise
                fbc_ps = psr_pool.tile([128, 36], F32, tag="pse", name="fbc_ps")
                nc.tensor.matmul(fbc_ps[:, :], onesrow[:, :], filtrow[:, :])
                fbc = spool.tile([128, 36], F32, tag="fbc")
                nc.scalar.activation(fbc[:, :], fbc_ps[:, :],
                                     mybir.ActivationFunctionType.Copy)
                pbc_ps = psr_pool.tile([128, 16], F32, tag="pse", name="pbc_ps")
                nc.tensor.matmul(pbc_ps[:, :], onesrow[:, :], pr[0:1, 0:16])
                bp = spool.tile([128, 16], F32, tag="bp")
                nc.vector.tensor_mul(bp[:, :], bctc_t[0:128, 32:48], pbc_ps[:, :])


            # main conv + evac per channel; tridiag builds interleaved so
            # the DVE FIFO isn't clogged ahead of the evacuations.
            ot = opool.tile([86, 3 * CPC * W], BF16, tag="ot")
            ot4 = ot[:, :].rearrange("p (c w x) -> p c w x", w=3, x=W)
            traw = {}
            tc0 = {}
            with nc.allow_low_precision(reason="bf16 conv pipeline"):
                for ch in range(CPC):
                    g = ch // 4
                    if ch % 4 == 0:
                        for dxi, dx in enumerate((-1, 0, 1)):
                            wm = 9 * g + (dx + 1)
                            w0c = wm + 3
                            wp = wm + 6
                            t1 = tmppool.tile([NPART, 86], BF16, tag="t1")
                            nc.vector.tensor_scalar_mul(
                                t1[:, :], shifts_t[0:NPART, 0:86], fbc[0:NPART, wm:wm + 1]
                            )
                            t2 = tmppool.tile([NPART, 86], BF16, tag="t2")
                            nc.vector.scalar_tensor_tensor(
                                t2[:, :], shifts_t[0:NPART, 1:87], fbc[0:NPART, w0c:w0c + 1],
                                t1[:, :], op0=mybir.AluOpType.mult, op1=mybir.AluOpType.add,
                            )
                            tr = tripool.tile([NPART, 86], BF16, tag=f"traw{g}_{dxi}")
                            nc.vector.scalar_tensor_tensor(
                                tr[:, :], shifts_t[0:NPART, 2:88], fbc[0:NPART, wp:wp + 1],
                                t2[:, :], op0=mybir.AluOpType.mult, op1=mybir.AluOpType.add,
                            )
                            traw[(g, dxi)] = tr
                    t = tripool.tile([NPART, 86], BF16, tag=f"tc0_{ch}")
                    nc.vector.scalar_tensor_tensor(
                        t[:, :], shifts_t[0:NPART, 1:87],
                        bctc_t[0:NPART, 16 + ch:16 + ch + 1],
                        traw[(g, 1)][:, :], op0=mybir.AluOpType.mult,
                        op1=mybir.AluOpType.add,
                    )
                    tc0[ch] = t
                    psa = psa_pool.tile([86, 512], F32, tag="psa")
                    if ch % 2 == 0:
                        psb2 = psb_pool.tile([85, 512], F32, tag="psb")
                        psb2_saved = psb2
                    else:
                        psb2 = psb2_saved
                    half = (ch % 2) * W
                    for dxi, dx in enumerate((-1, 0, 1)):
                        lt = tc0[ch] if dx == 0 else traw[(g, dxi)]
                        hh, c8 = divmod(ch, 8)
                        nc.tensor.matmul(
                            psa[:, :],
                            lt[0:NPART, 0:86],
                            t4[0:NPART, hh, 0:2, c8, dx + 1:dx + 257],
                            start=(dxi == 0),
                            stop=(dxi == 2),
                        )
                        if dx == 0:
                            nc.tensor.matmul(
                                psb2[0:85, half:half + W],
                                lt[0:87, 0:85],
                                t4[0:87, hh, 2, c8, dx + 1:dx + 257],
                                start=False,
                                stop=(ch % 2 == 1),
                            )
                        elif ch % 2 == 0:
                            # paired: w2 of ch and ch+1 share the group lhsT
                            nc.tensor.matmul(
                                psb2[0:85, :],
                                lt[0:87, 0:85],
                                t4[0:87, hh, 2, c8:c8 + 2, dx + 1:dx + 257],
                                start=(dxi == 0),
                                stop=False,
                            )
                    # psa evac alternates DVE / ScalarE to balance engines
                    if ch % 2 == 0:
                        nc.vector.tensor_scalar(
                            ot4[0:86, ch, 0:2, :],
                            psa[:, :].rearrange("p (a b) -> p a b", b=W),
                            bctc_t[0:86, ch:ch + 1],
                            bp[0:86, ch:ch + 1],
                            op0=mybir.AluOpType.mult,
                            op1=mybir.AluOpType.add,
                        )
                    else:
                        nc.scalar.activation(
                            ot4[0:86, ch, 0:2, :],
                            psa[:, :].rearrange("p (a b) -> p a b", b=W),
                            mybir.ActivationFunctionType.Identity,
                            bias=bp[0:86, ch:ch + 1],
                            scale=bctc_t[0:86, ch:ch + 1],
                        )
                    if ch % 2 == 1:
                        for c2 in (ch - 1, ch):
                            h2 = (c2 % 2) * W
                            nc.scalar.activation(
                                ot4[0:85, c2, 2, :], psb2[0:85, h2:h2 + W],
                                mybir.ActivationFunctionType.Identity,
                                bias=bp[0:85, c2:c2 + 1],
                                scale=bctc_t[0:85, c2:c2 + 1],
                            )
                        # out DMA per channel: smaller final transfer
                        for c2 in (ch - 1, ch):
                            c0 = c2 * OBLK
                            on_q(nc.gpsimd.dma_start(
                                out_d[0:86, c0:c0 + OBLK],
                                ot[0:86, c0:c0 + OBLK],
                            ), c2 % 4)

    nc.compile()
    return nc


_NC_CACHE = None


def _get_nc():
    global _NC_CACHE
    if _NC_CACHE is None:
        _NC_CACHE = _build_nc()
    return _NC_CACHE


def _maskcol_np():
    s = np.zeros((NPART, 4), np.float32)
    for wi, win in enumerate(WINDOWS):
        pv0, pv1 = win[5], win[6]
        s[pv0:pv1, wi] = 1.0
    return s.astype(NPBF16)


def _shifts_np():
    return np.eye(NPART, dtype=np.float32).astype(NPBF16)


# row indices per window (length NPART; tail rows unused -> clamp to 0)
def _win_rows():
    rows = []
    for wi, (r0, nr, p0, rr, rp, _, _, _, _, nparts) in enumerate(WINDOWS):
        idx = np.zeros(NPART, np.int64)
        idx[p0:p0 + nr] = np.arange(r0, r0 + nr)
        if rr is not None:
            idx[rp] = rr
        rows.append((idx, nparts))
    return rows


_WIN_ROWS = _win_rows()


def _build_windows(xs_np):
    """xs_np [16, 256, 256] fp32 -> [NPART, (h2, w3, c8, 264)] bf16 windows."""
    out = np.zeros((NPART, 3, CPC, WBLK), NPBF16)
    xb = xs_np.astype(NPBF16)
    for wi, (idx, nparts) in enumerate(_WIN_ROWS):
        g = xb[:, idx[:nparts], :]             # [16, nparts, 256]
        g = np.ascontiguousarray(g.transpose(1, 0, 2))  # [nparts, 16, 256]
        out[:nparts, wi, :, 1:257] = g
        out[:nparts, wi, :, 0] = g[:, :, 1]
        out[:nparts, wi, :, 257] = g[:, :, 254]
    out = out.reshape(NPART, 3, 2, 8, WBLK).transpose(0, 2, 1, 3, 4)
    return np.ascontiguousarray(out).reshape(NPART, 3 * CPC * WBLK)


def _build_xe(xs_np):
    """xs_np [16, 256, 256] fp32 -> [128, (t=2, x=256, c=16)] fp8."""
    r = xs_np.reshape(CPC, 2, 128, 256).transpose(2, 1, 3, 0)  # [128, 2, 256, 16]
    return np.ascontiguousarray(r).astype(NPFP8).reshape(128, XEC)


def _scatter_out(flat, dst):
    """flat [86, 16*3*256] bf16 (c, w, x) -> dst [16, 256, 256] fp32."""
    f = flat.astype(np.float32).reshape(86, CPC, 3, W)
    dst[:, 0:86, :] = f[0:86, :, 0].transpose(1, 0, 2)
    dst[:, 86:171, :] = f[0:85, :, 1].transpose(1, 0, 2)
    dst[:, 171:256, :] = f[0:85, :, 2].transpose(1, 0, 2)


def kernel(x, conv_w, bn_gamma, bn_beta, bn_mean, bn_var, lamb_l, lamb_h, inside_all):
    x = np.asarray(x, np.float32)
    conv_w = np.asarray(conv_w, np.float32)
    bn_gamma = np.asarray(bn_gamma, np.float32)
    bn_beta = np.asarray(bn_beta, np.float32)
    bn_mean = np.asarray(bn_mean, np.float32)
    bn_var = np.asarray(bn_var, np.float32)
    lamb_l = np.asarray(lamb_l, np.float32)
    lamb_h = np.asarray(lamb_h, np.float32)
    ia = np.asarray(inside_all, np.float32).reshape(C)

    gv = (bn_gamma / np.sqrt(bn_var + np.float32(EPS))).astype(np.float32)
    w2s = (conv_w * gv[:, None] / np.float32(H * W)).astype(np.float32)  # [72, 32]
    b2 = (bn_beta - bn_mean * gv).astype(np.float32)                      # [72]

    A = (lamb_l * (ia + 1.0)).astype(np.float32)
    s = (lamb_h + 1.0).astype(np.float32)
    # device bias multiplies Bc by the pooled SUM, so fold the mean's 1/HW here
    Bc = (-lamb_l * ia / np.float32(H * W)).astype(np.float32)
    A_eff = np.where(A >= 0, np.maximum(A, 1e-20), np.minimum(A, -1e-20)).astype(np.float32)
    sig = (s / A_eff).astype(np.float32)

    shifts = _shifts_np()
    maskcol = _maskcol_np()
    nc = _get_nc()

    in_maps = []
    for core in range(NCORES):
        n = core // 2
        half = core % 2
        csl = slice(16 * half, 16 * half + 16)
        osl = slice(16 * (1 - half), 16 * (1 - half) + 16)
        gsl = slice(36 * half, 36 * half + 36)
        bctc_row = np.concatenate([A_eff[csl], sig[csl], Bc[csl]]).astype(np.float32)
        w2sT = np.ascontiguousarray(w2s[gsl].T)          # [32 in-ch, 36]
        w2t_full = np.concatenate(
            [w2sT[csl], w2sT[osl], b2[gsl].reshape(1, 36)], axis=0
        ).astype(NPBF16)                                  # [33, 36]
        in_maps.append({
            "xw": _build_windows(x[n, csl]),
            "xo": _build_xe(x[n, csl]),
            "xe": _build_xe(x[n, osl]),
            "w2t": w2t_full,
            "bctc": np.tile(bctc_row[None, :], (128, 1)),
            "shifts": shifts,
            "maskcol": maskcol,
        })

    res = bass_utils.run_bass_kernel_spmd(nc, in_maps, core_ids=list(range(NCORES)))

    out = np.empty((N_B, C, H, W), np.float32)
    for core in range(NCORES):
        n = core // 2
        half = core % 2
        _scatter_out(res.results[core]["out"], out[n, 16 * half:16 * half + 16])
    return out
